# revision 5
# baseline (speedup 1.0000x reference)
"""BatchSpecAugment on 8 Trainium2 NeuronCores.

Strategy: the output is `mel` with per-sample rectangular regions zeroed
(2 random time-mask row intervals + 2 random freq-mask column bands over
the valid time region).  The mask geometry depends only on `lengths` and
a fixed PRNG key, so it is computed on host (tiny [64,2] arrays; JAX
threefry is backend-deterministic) and shipped to the device as three
small per-sample vectors.  The device kernel is a pure memory-streaming
pass, data-parallel over batch (8 samples per core):

    out[t, f] = mel[t, f] * rA[t] - (mel[t, f] * rB[t]) * c[f]

where rA[t] = 1 - time_mask[t], rB[t] = rA[t] * (t < len), c[f] = freq
mask indicator.  rA/rB enter as per-partition scalars (t on partitions),
c as a partition-broadcast tile, so each 128x128 block needs two fused
DVE ops (scalar_tensor_tensor).  All values are exactly 0.0/1.0 so the
arithmetic is bit-exact (x*1=x, x-x=+0, x-0=x).
"""
import sys

sys.path.insert(0, "/opt/trn_rl_repo")

import numpy as np

B, T, F = 64, 4000, 128
N_CORES = 8
SPB = B // N_CORES          # samples per core
TIME_MASK_PARAM = 40
FREQ_MASK_PARAM = 30
NUM_TIME_MASKS = 2
NUM_FREQ_MASKS = 2
NBLK = 32                   # ceil(4000/128): 31 full 128-row blocks + one 32-row block
PART_ROWS = T - 31 * 128    # 32 rows in the last block
# (start_block, n_blocks) groups for the 31 full blocks; one DMA per group
GROUPS = [(0, 8), (8, 8), (16, 8), (24, 7)]


def _host_mask_params(lengths: np.ndarray):
    """Reproduce the reference's random draws exactly.

    Must run on the DEFAULT jax backend: this environment's PRNG impl is
    `rbg`, whose draws are backend-dependent, and the grading reference
    runs on the default (neuron) backend — forcing CPU here would give
    different random bits.
    """
    import jax
    import jax.numpy as jnp

    key = jax.random.key(42)
    kf_w, kf_s, kt_w, kt_s = jax.random.split(key, 4)
    valid_t = jnp.asarray(lengths).astype(jnp.int32)

    fw_max = min(FREQ_MASK_PARAM, F)
    fwidths = jax.random.randint(kf_w, (B, NUM_FREQ_MASKS), 0, fw_max + 1)
    fu = jax.random.uniform(kf_s, (B, NUM_FREQ_MASKS))
    fstarts = jnp.floor(fu * (F - fwidths + 1).astype(jnp.float32)).astype(jnp.int32)
    fvalid = (fwidths > 0) & (fwidths < F)

    tw_max = jnp.minimum(TIME_MASK_PARAM, valid_t)
    tu = jax.random.uniform(kt_w, (B, NUM_TIME_MASKS))
    twidths = jnp.floor(tu * (tw_max[:, None] + 1).astype(jnp.float32)).astype(jnp.int32)
    twidths = jnp.minimum(twidths, tw_max[:, None])
    su = jax.random.uniform(kt_s, (B, NUM_TIME_MASKS))
    srange = jnp.maximum(valid_t[:, None] - twidths + 1, 1)
    tstarts = jnp.floor(su * srange.astype(jnp.float32)).astype(jnp.int32)
    tvalid = (twidths > 0) & (twidths < valid_t[:, None]) & (valid_t[:, None] > 0)

    return (np.asarray(fstarts), np.asarray(fwidths), np.asarray(fvalid),
            np.asarray(tstarts), np.asarray(twidths), np.asarray(tvalid),
            np.asarray(valid_t))


def _build_device_inputs(lengths: np.ndarray):
    """Build per-core rab [128, SPB*64] and cbb [128, SPB*128] f32 tensors.

    rab[p, s*64 + k]      = rA of row 128k+p of local sample s
    rab[p, s*64 + 32 + k] = rB of row 128k+p
    cbb[p, s*128 + f]     = c[f] of local sample s (partition-broadcast)
    """
    fstarts, fwidths, fvalid, tstarts, twidths, tvalid, valid_t = _host_mask_params(lengths)

    TP = NBLK * 128  # padded T = 4096
    rA = np.ones((B, TP), dtype=np.float32)
    rB = np.zeros((B, TP), dtype=np.float32)
    cf = np.zeros((B, F), dtype=np.float32)
    t_idx = np.arange(TP)
    for b in range(B):
        for i in range(NUM_TIME_MASKS):
            if tvalid[b, i]:
                rA[b, tstarts[b, i]: tstarts[b, i] + twidths[b, i]] = 0.0
        rB[b] = rA[b] * (t_idx < valid_t[b])
        for i in range(NUM_FREQ_MASKS):
            if fvalid[b, i]:
                cf[b, fstarts[b, i]: fstarts[b, i] + fwidths[b, i]] = 1.0

    rabs, cbbs = [], []
    for core in range(N_CORES):
        rab = np.zeros((128, SPB * 64), dtype=np.float32)
        cbb = np.zeros((128, SPB * F), dtype=np.float32)
        for s in range(SPB):
            b = core * SPB + s
            rab[:, s * 64: s * 64 + 32] = rA[b].reshape(NBLK, 128).T
            rab[:, s * 64 + 32: s * 64 + 64] = rB[b].reshape(NBLK, 128).T
            cbb[:, s * F: (s + 1) * F] = cf[b][None, :]
        rabs.append(rab)
        cbbs.append(cbb)
    return rabs, cbbs


_PROGRAM_CACHE = {}


def _build_program(loop_n: int = 1):
    """Build the SPMD program. loop_n > 1 wraps the body in an on-device
    For_i loop (identical idempotent iterations) — used only for timing
    via wall-clock differencing."""
    if loop_n in _PROGRAM_CACHE:
        return _PROGRAM_CACHE[loop_n]

    from contextlib import ExitStack

    import concourse.bass as bass
    import concourse.tile as tile
    from concourse import bacc, mybir

    f32 = mybir.dt.float32
    mult = mybir.AluOpType.mult
    subtract = mybir.AluOpType.subtract

    nc = bacc.Bacc("TRN2", target_bir_lowering=False, debug=False,
                   num_devices=N_CORES)

    mel = nc.dram_tensor("mel", [SPB, T, F], f32, kind="ExternalInput").ap()
    rab = nc.dram_tensor("rab", [128, SPB * 64], f32, kind="ExternalInput").ap()
    cbb = nc.dram_tensor("cbb", [128, SPB * F], f32, kind="ExternalInput").ap()
    out = nc.dram_tensor("out", [SPB, T, F], f32, kind="ExternalOutput").ap()

    with tile.TileContext(nc) as tc, ExitStack() as ctx:
        const_pool = ctx.enter_context(tc.tile_pool(name="const", bufs=1))
        mel_pool = ctx.enter_context(tc.tile_pool(name="mel", bufs=4))
        u_pool = ctx.enter_context(tc.tile_pool(name="u", bufs=8))

        rab_sb = const_pool.tile([128, SPB * 64], f32, tag="rab")
        nc.sync.dma_start(out=rab_sb[:], in_=rab[:])
        cbb_sb = const_pool.tile([128, SPB * F], f32, tag="cbb")
        nc.sync.dma_start(out=cbb_sb[:], in_=cbb[:])

        def process(s, g0, nb, rows):
            """One DMA group: blocks g0..g0+nb-1 of sample s, `rows` rows each."""
            t = mel_pool.tile([rows, nb * 128], f32, tag="mel")
            src = mel[s, g0 * 128: g0 * 128 + (nb - 1) * 128 + rows, :]
            if nb > 1:
                src = src.rearrange("(j p) f -> p j f", p=128)
                nc.sync.dma_start(out=t[:].rearrange("p (j f) -> p j f", f=F), in_=src)
            else:
                nc.sync.dma_start(out=t[:], in_=src)
            for j in range(nb):
                k = g0 + j
                blk = t[:, bass.ts(j, 128)]
                ia = s * 64 + k
                ib = ia + 32
                u = u_pool.tile([rows, 128], f32, tag="u")
                # u = (mel * rB[p]) * c
                nc.vector.scalar_tensor_tensor(
                    out=u[:], in0=blk, scalar=rab_sb[:rows, ib:ib + 1],
                    in1=cbb_sb[:rows, s * F:(s + 1) * F], op0=mult, op1=mult)
                # out = (mel * rA[p]) - u
                nc.vector.scalar_tensor_tensor(
                    out=blk, in0=blk, scalar=rab_sb[:rows, ia:ia + 1],
                    in1=u[:], op0=mult, op1=subtract)
            dst = out[s, g0 * 128: g0 * 128 + (nb - 1) * 128 + rows, :]
            if nb > 1:
                dst = dst.rearrange("(j p) f -> p j f", p=128)
                nc.scalar.dma_start(out=dst, in_=t[:].rearrange("p (j f) -> p j f", f=F))
            else:
                nc.scalar.dma_start(out=dst, in_=t[:])

        def body():
            for s in range(SPB):
                for g0, nb in GROUPS:
                    process(s, g0, nb, 128)
                process(s, 31, 1, PART_ROWS)

        if loop_n == 1:
            body()
        else:
            hint = tuple(e for n in ("DVE", "SP", "Activation")
                         if (e := getattr(mybir.EngineType, n, None)) is not None)
            with tc.For_i(0, loop_n, 1, hint_engines=hint):
                body()

    nc.compile()
    _PROGRAM_CACHE[loop_n] = nc
    return nc


def kernel(mel: np.ndarray, lengths: np.ndarray) -> np.ndarray:
    from concourse.bass_utils import run_bass_kernel_spmd

    mel = np.ascontiguousarray(np.asarray(mel), dtype=np.float32)
    lengths = np.asarray(lengths)
    rabs, cbbs = _build_device_inputs(lengths)

    nc = _build_program()
    in_maps = [
        {"mel": mel[core * SPB:(core + 1) * SPB], "rab": rabs[core], "cbb": cbbs[core]}
        for core in range(N_CORES)
    ]
    res = run_bass_kernel_spmd(nc, in_maps, core_ids=list(range(N_CORES)))
    return np.concatenate([np.asarray(r["out"]) for r in res.results], axis=0)
